# revision 14
# baseline (speedup 1.0000x reference)
"""Trainium2 Bass kernel for nn_ExLoss (exclusive CE + batch/class hard mining).

Strategy (tensor parallel over classes, 8 NeuronCores):
  - V is sharded along the class axis: core i owns classes [i*2048, (i+1)*2048).
    The host feeds each core V_shard.T (layout [D, C_local]) so the matmul's
    moving operand streams directly from DRAM without on-chip transposition.
  - outputs = inputs @ V.T runs in float32r (full-rate PE) per shard;
    tsims[c,b] = outputs[b,c]/|x_b| (T=1.0) comes from PE transposes of the
    output tiles plus a column scale — no second big matmul.
  - log-sum-exp / target-logit / per-class th-loss partials are reduced per
    core, merged with one tiny AllGather ([1,1024] per core); every core
    computes the identical final scalar loss.
  - The batch-internal (h) path is tiny ([256,256]) and computed redundantly
    on every core from G = inputs @ inputs.T (fp32r), with |x| from diag(G).
"""

import numpy as np

import concourse.bass as bass
import concourse.mybir as mybir
import concourse.tile as tile
from concourse import bacc, bass_isa, bass_utils
from concourse.hw_specs import get_activation_tables as _gat
from concourse.masks import make_identity

F32 = mybir.dt.float32
F32R = mybir.dt.float32r
U8 = mybir.dt.uint8
AF = mybir.ActivationFunctionType
OP = mybir.AluOpType
AX = mybir.AxisListType

NCORES = 8
B = 256
D = 2048
C = 16384
CL = C // NCORES          # 2048 classes per core
GW = 512                  # class group width (one PSUM bank)
NG = CL // GW             # 4 groups
KC = D // 128             # 16 contraction chunks
MARGIN = 0.3
BIG = 1e9

_ACT_TABLE = "natural_log_exp_and_others"


def _pinned_act_tables(arch):
    # Keep list order/indices (act_func_set_id indexes act_info.json), but
    # blank every table except the one holding Exp/Ln/Copy/Identity/Square so
    # bacc's table-load pass can't thrash between per-function tables.
    return {name: (funcs if name == _ACT_TABLE else set())
            for name, funcs in _gat(arch).items()}


bacc.get_activation_tables = _pinned_act_tables

# build-time knobs (model-tuned)
CFG = {
    "vt_kj": 8,            # k-chunks per VT DMA (8 -> 2MB chunks)
    "h_early": True,       # emit batch-internal path before the main loop
    "th_pergroup": False,  # finalize th per group vs at the tail
    "xt_on_act": False,    # xt/small input DMAs on the ACT HWDGE ring
}


def build():
    nc = bacc.Bacc(None, target_bir_lowering=False, debug=False, num_devices=NCORES)

    xT_d = nc.dram_tensor("xT", [D, B], F32R, kind="ExternalInput")
    vT_d = nc.dram_tensor("vT", [D, CL], F32R, kind="ExternalInput")
    trow_d = nc.dram_tensor("trow", [1, B], F32, kind="ExternalInput")
    tcol_d = nc.dram_tensor("tcol", [B, 1], F32, kind="ExternalInput")
    posm_d = nc.dram_tensor("posm", [B, B], U8, kind="ExternalInput")
    negm_d = nc.dram_tensor("negm", [B, B], U8, kind="ExternalInput")
    out_d = nc.dram_tensor("out", [B, CL], F32, kind="ExternalOutput")
    loss_d = nc.dram_tensor("loss", [1, 1], F32, kind="ExternalOutput")

    in_eng = nc.scalar if CFG["xt_on_act"] else nc.sync

    with tile.TileContext(nc) as tc:
        with (
            tc.tile_pool(name="const", bufs=1) as cpool,
            tc.tile_pool(name="vt", bufs=3) as vtpool,
            tc.tile_pool(name="grp", bufs=3) as gpool,
            tc.tile_pool(name="slab", bufs=2) as spool,
            tc.tile_pool(name="small", bufs=1) as smp,
            tc.tile_pool(name="acc", bufs=4, space="PSUM") as accp,
            tc.tile_pool(name="tp", bufs=4, space="PSUM") as tpp,
            tc.tile_pool(name="dram", bufs=1, space="DRAM") as dram,
        ):
            # ---------------- constants / small inputs ----------------
            ident = cpool.tile([128, 128], F32, tag="ident")
            make_identity(nc, ident[:])
            onesf = cpool.tile([128, 1], F32, tag="ones")
            nc.vector.memset(onesf[:], 1.0)
            bneg4 = cpool.tile([128, 1], F32, tag="bneg4")
            nc.vector.memset(bneg4[:], -4.0)

            iota512 = cpool.tile([128, GW], F32, tag="iota512")
            nc.gpsimd.iota(iota512[:], pattern=[[1, GW]], base=0,
                           channel_multiplier=0, allow_small_or_imprecise_dtypes=True)
            cls_idx = cpool.tile([128, KC], F32, tag="clsidx")
            nc.gpsimd.iota(cls_idx[:], pattern=[[128, KC]], base=0,
                           channel_multiplier=1, allow_small_or_imprecise_dtypes=True)
            dmi = cpool.tile([128, 128], F32, tag="dmi")
            nc.gpsimd.iota(dmi[:], pattern=[[1, 128]], base=0,
                           channel_multiplier=-1, allow_small_or_imprecise_dtypes=True)
            diagmask = cpool.tile([128, 128], F32, tag="diagmask")
            nc.vector.tensor_scalar(diagmask[:], dmi[:], 0.0, None, OP.is_equal)

            trow = cpool.tile([1, B], F32, tag="trow")
            in_eng.dma_start(out=trow[:], in_=trow_d[:, :])
            t_rep = cpool.tile([128, B], F32, tag="trep")
            nc.gpsimd.partition_broadcast(t_rep[:], trow[:], channels=128)

            tcol = [cpool.tile([128, 1], F32, tag=f"tcol{m}", name=f"tcol{m}")
                    for m in range(2)]
            for m in range(2):
                in_eng.dma_start(out=tcol[m][:], in_=tcol_d[m * 128:(m + 1) * 128, 0:1])

            posf = [cpool.tile([128, B], F32, tag=f"posf{m}", name=f"posf{m}")
                    for m in range(2)]
            negf = [cpool.tile([128, B], F32, tag=f"negf{m}", name=f"negf{m}")
                    for m in range(2)]
            for m in range(2):
                nc.gpsimd.dma_start(out=posf[m][:], in_=posm_d[m * 128:(m + 1) * 128, :])
                nc.gpsimd.dma_start(out=negf[m][:], in_=negm_d[m * 128:(m + 1) * 128, :])

            # xT resident: [128, KC, B] (f32r)
            xt = cpool.tile([128, KC, B], F32R, tag="xt")
            in_eng.dma_start(out=xt[:],
                             in_=xT_d.ap().rearrange("(k p) b -> p k b", p=128))

            # ---------------- G = x @ x.T (fp32r), norms, r ----------------
            G_sb = [cpool.tile([128, B], F32, tag=f"gsb{m}", name=f"gsb{m}")
                    for m in range(2)]
            rinv = [cpool.tile([128, 1], F32, tag=f"rinv{m}", name=f"rinv{m}")
                    for m in range(2)]
            rrow = cpool.tile([1, B], F32, tag="rrow")
            for m in range(2):
                g_ps = tpp.tile([128, B], F32, tag="tp", name="g_ps")
                for k in range(KC):
                    nc.tensor.matmul(g_ps[:], xt[:, k, m * 128:(m + 1) * 128],
                                     xt[:, k, :], start=(k == 0), stop=(k == KC - 1))
                nc.scalar.copy(G_sb[m][:], g_ps[:])
                scrd = smp.tile([128, 128], F32, tag="scrd", name="scrd")
                nsq = smp.tile([128, 1], F32, tag=f"nsq{m}", name=f"nsq{m}")
                nc.vector.scalar_tensor_tensor(
                    scrd[:], diagmask[:], 1.0, G_sb[m][:, m * 128:(m + 1) * 128],
                    OP.mult, OP.mult, accum_out=nsq[:])
                lnn = smp.tile([128, 1], F32, tag=f"lnn{m}", name=f"lnn{m}")
                nc.scalar.activation(lnn[:], nsq[:], AF.Ln)
                nc.scalar.activation(rinv[m][:], lnn[:], AF.Exp, scale=-0.5)
                rt_ps = tpp.tile([128, B], F32, tag="tp", name="rt_ps")
                nc.tensor.transpose(rt_ps[0:1, 0:128], rinv[m][:], ident[:])
                nc.scalar.copy(rrow[0:1, m * 128:(m + 1) * 128], rt_ps[0:1, 0:128])
            r_rep = cpool.tile([128, B], F32, tag="rrep")
            nc.gpsimd.partition_broadcast(r_rep[:], rrow[:], channels=128)

            def h_path():
                # batch-internal mining on sim = G * r_i * r_j (tiny, redundant
                # on every core). Returns the [1,1] PSUM partition-sum.
                h_ps = tpp.tile([1, 1], F32, tag="tp", name="h_ps")
                for m in range(2):
                    simm = smp.tile([128, B], F32, tag=f"sim{m}", name=f"sim{m}")
                    nc.scalar.mul(simm[:], G_sb[m][:], rinv[m][:])
                    nc.vector.tensor_tensor(out=simm[:], in0=simm[:], in1=r_rep[:],
                                            op=OP.mult)
                    scrh = smp.tile([128, B], F32, tag="scrh", name="scrh")
                    minzp = smp.tile([128, 1], F32, tag="minzp", name="minzp")
                    nc.vector.scalar_tensor_tensor(scrh[:], posf[m][:], -4.0, simm[:],
                                                   OP.mult, OP.add)
                    nc.vector.tensor_reduce(minzp[:], scrh[:], axis=AX.X, op=OP.min)
                    hasp = smp.tile([128, 1], F32, tag="hasp", name="hasp")
                    nc.vector.tensor_scalar(hasp[:], minzp[:], -2.0, None, OP.is_lt)
                    hasp_u8 = smp.tile([128, 1], U8, tag="haspu8", name="hasp_u8")
                    nc.vector.tensor_scalar(hasp_u8[:], minzp[:], -2.0, None, OP.is_lt)
                    thrd_h = smp.tile([128, 1], F32, tag="thrdh", name="thrd_h")
                    tta_h = smp.tile([128, 1], F32, tag="ttah", name="tta_h")
                    nc.vector.tensor_scalar(tta_h[:], minzp[:], 4.0 - MARGIN, None,
                                            OP.add)
                    nc.vector.memset(thrd_h[:], BIG)
                    nc.vector.copy_predicated(thrd_h[:], hasp_u8[:], tta_h[:])
                    hpl = smp.tile([128, 1], F32, tag="hpl", name="hpl")
                    nc.scalar.activation(hpl[:], minzp[:], AF.Exp, bias=bneg4[:],
                                         scale=-1.0)
                    nc.vector.tensor_scalar(hpl[:], hpl[:], 1.0, None, OP.add)
                    nc.scalar.activation(hpl[:], hpl[:], AF.Ln)
                    nc.vector.tensor_tensor(out=hpl[:], in0=hpl[:], in1=hasp[:],
                                            op=OP.mult)
                    # negatives: zn = sim + 4*neg; threshold shifted by 4
                    # (f32-safe; thrd=BIG rows select nothing since zn <= 5)
                    nc.vector.scalar_tensor_tensor(scrh[:], negf[m][:], 4.0, simm[:],
                                                   OP.mult, OP.add)
                    ths = smp.tile([128, 1], F32, tag="ths", name="ths")
                    nc.vector.tensor_scalar(ths[:], thrd_h[:], 4.0, None, OP.add)
                    sph = smp.tile([128, B], F32, tag="sph", name="sph")
                    nc.scalar.activation(sph[:], simm[:], AF.Exp)
                    nc.vector.tensor_scalar(sph[:], sph[:], 1.0, None, OP.add)
                    nc.scalar.activation(sph[:], sph[:], AF.Ln)
                    hcnt = smp.tile([128, 1], F32, tag="hcnt", name="hcnt")
                    hscr = smp.tile([128, B], F32, tag="hscr", name="hscr")
                    nc.vector.tensor_scalar(hscr[:], scrh[:], ths[:], None, OP.is_gt,
                                            OP.add, accum_out=hcnt[:])
                    hnsum = smp.tile([128, 1], F32, tag="hnsum", name="hnsum")
                    nc.vector.scalar_tensor_tensor(hscr[:], scrh[:], ths[:], sph[:],
                                                   OP.is_gt, OP.mult,
                                                   accum_out=hnsum[:])
                    cmax = smp.tile([128, 1], F32, tag="cmax", name="cmax")
                    nc.vector.tensor_scalar(cmax[:], hcnt[:], 1.0, None, OP.max)
                    rec = smp.tile([128, 1], F32, tag="rec", name="rec")
                    nc.vector.reciprocal(rec[:], cmax[:])
                    cm = smp.tile([128, 1], F32, tag="cm", name="cm")
                    nc.vector.tensor_scalar(cm[:], hcnt[:], 0.5, None, OP.is_gt)
                    nc.vector.tensor_tensor(out=hnsum[:], in0=hnsum[:], in1=rec[:],
                                            op=OP.mult)
                    nc.vector.tensor_tensor(out=hnsum[:], in0=hnsum[:], in1=cm[:],
                                            op=OP.mult)
                    hm = smp.tile([128, 1], F32, tag="hm", name="hm")
                    nc.vector.tensor_tensor(out=hm[:], in0=hpl[:], in1=hnsum[:],
                                            op=OP.add)
                    nc.tensor.matmul(h_ps[:], hm[:], onesf[:],
                                     start=(m == 0), stop=(m == 1))
                return h_ps

            if CFG["h_early"]:
                h_ps = h_path()

            # ---------------- main loop over class groups ----------------
            rowmax = [smp.tile([128, NG], F32, tag=f"rowmax{m}", name=f"rowmax{m}")
                      for m in range(2)]
            sexp = [smp.tile([128, NG], F32, tag=f"sexp{m}", name=f"sexp{m}")
                    for m in range(2)]
            tgat = [smp.tile([128, NG], F32, tag=f"tgat{m}", name=f"tgat{m}")
                    for m in range(2)]
            if CFG["th_pergroup"]:
                thacc = smp.tile([128, 1], F32, tag="thacc")
            else:
                minz_all = smp.tile([128, KC], F32, tag="minz")
                tcnt_all = smp.tile([128, KC], F32, tag="tcnt")
                thn_all = smp.tile([128, KC], F32, tag="thn")

            KJ = CFG["vt_kj"]
            for g in range(NG):
                acc = [accp.tile([128, GW], F32, tag="acc", name=f"acc{_m}")
                       for _m in range(2)]
                for kk in range(KC // KJ):
                    vt = vtpool.tile([128, KJ, GW], F32R, tag="vt", name="vt")
                    nc.sync.dma_start(
                        out=vt[:],
                        in_=vT_d[kk * KJ * 128:(kk + 1) * KJ * 128,
                                 g * GW:(g + 1) * GW]
                        .rearrange("(j p) c -> p j c", p=128))
                    for j in range(KJ):
                        k = kk * KJ + j
                        for m in range(2):
                            nc.tensor.matmul(acc[m][:],
                                             xt[:, k, m * 128:(m + 1) * 128],
                                             vt[:, j, :], start=(k == 0),
                                             stop=(k == KC - 1))

                outa = [gpool.tile([128, GW], F32, tag=f"outa{m}", name=f"outa{m}")
                        for m in range(2)]
                for m in range(2):
                    nc.scalar.copy(outa[m][:], acc[m][:])
                    nc.sync.dma_start(out=out_d[m * 128:(m + 1) * 128,
                                                g * GW:(g + 1) * GW],
                                      in_=outa[m][:])
                    nc.vector.tensor_reduce(rowmax[m][:, g:g + 1], outa[m][:],
                                            axis=AX.X, op=OP.max)
                    negmax = gpool.tile([128, 1], F32, tag=f"negmax{m}",
                                        name=f"negmax{m}")
                    nc.vector.tensor_scalar(negmax[:], rowmax[m][:, g:g + 1], -1.0,
                                            None, OP.mult)
                    escr = gpool.tile([128, GW], F32, tag=f"escr{m}", name="escr")
                    nc.scalar.activation(escr[:], outa[m][:], AF.Exp,
                                         bias=negmax[:], accum_out=sexp[m][:, g:g + 1])
                    tadj = gpool.tile([128, 1], F32, tag=f"tadj{m}", name=f"tadj{m}")
                    nc.vector.tensor_scalar(tadj[:], tcol[m][:], float(-g * GW),
                                            None, OP.add)
                    gscr = gpool.tile([128, GW], F32, tag=f"gscr{m}", name="gscr")
                    nc.vector.scalar_tensor_tensor(
                        gscr[:], iota512[:], tadj[:], outa[m][:],
                        OP.is_equal, OP.mult, accum_out=tgat[m][:, g:g + 1])

                # transpose group -> [class, b], scale columns by 1/|x_b|
                outbT = spool.tile([128, 4, B], F32, tag="outbT")
                for q in range(4):
                    tp_ps = tpp.tile([128, B], F32, tag="tp", name="tp_ps")
                    for m in range(2):
                        nc.tensor.transpose(tp_ps[:, m * 128:(m + 1) * 128],
                                            outa[m][:, q * 128:(q + 1) * 128],
                                            ident[:])
                    nc.scalar.copy(outbT[:, q, :], tp_ps[:])
                tsims = spool.tile([128, 4, B], F32, tag="tsims")
                nc.vector.tensor_tensor(
                    out=tsims[:], in0=outbT[:],
                    in1=r_rep[:].unsqueeze(1).broadcast_to([128, 4, B]), op=OP.mult)

                tpos4 = spool.tile([128, 4, B], F32, tag="tpos4")
                for q in range(4):
                    nc.vector.tensor_scalar(tpos4[:, q, :], t_rep[:],
                                            cls_idx[:, g * 4 + q:g * 4 + q + 1], 4.0,
                                            OP.is_equal, OP.mult)
                z = spool.tile([128, 4, B], F32, tag="z")
                nc.vector.scalar_tensor_tensor(z[:], tpos4[:], -1.0, tsims[:],
                                               OP.mult, OP.add)
                if CFG["th_pergroup"]:
                    minz_g = spool.tile([128, 4], F32, tag="minzg", name="minz_g")[:]
                else:
                    minz_g = minz_all[:, g * 4:(g + 1) * 4]
                nc.vector.tensor_reduce(minz_g, z[:], axis=AX.X, op=OP.min)
                sp = spool.tile([128, 4, B], F32, tag="sp")
                nc.scalar.activation(sp[:], tsims[:], AF.Exp)
                nc.vector.tensor_scalar(sp[:], sp[:], 1.0, None, OP.add)
                nc.scalar.activation(sp[:], sp[:], AF.Ln)

                thrd = spool.tile([128, 4], F32, tag="thrd")
                hasm = spool.tile([128, 4], F32, tag="hasm")
                nc.vector.tensor_scalar(hasm[:], minz_g, -2.0, None, OP.is_lt)
                hasm_u8 = spool.tile([128, 4], U8, tag="hasmu8")
                nc.vector.tensor_scalar(hasm_u8[:], minz_g, -2.0, None, OP.is_lt)
                tta = spool.tile([128, 4], F32, tag="tta")
                nc.vector.tensor_scalar(tta[:], minz_g, 4.0 - MARGIN, None, OP.add)
                nc.vector.memset(thrd[:], 1.0 - MARGIN)
                nc.vector.copy_predicated(thrd[:], hasm_u8[:], tta[:])

                scr1 = spool.tile([128, 4, B], F32, tag="scr1")
                scr2 = spool.tile([128, 4, B], F32, tag="scr2")
                if CFG["th_pergroup"]:
                    tcnt_g = spool.tile([128, 4], F32, tag="tcntg", name="tcnt_g")
                    thn_g = spool.tile([128, 4], F32, tag="thng", name="thn_g")
                for q in range(4):
                    cslice = (tcnt_g[:, q:q + 1] if CFG["th_pergroup"]
                              else tcnt_all[:, g * 4 + q:g * 4 + q + 1])
                    nslice = (thn_g[:, q:q + 1] if CFG["th_pergroup"]
                              else thn_all[:, g * 4 + q:g * 4 + q + 1])
                    nc.vector.tensor_scalar(scr1[:, q, :], z[:, q, :],
                                            thrd[:, q:q + 1], None, OP.is_gt, OP.add,
                                            accum_out=cslice)
                    nc.vector.scalar_tensor_tensor(
                        scr2[:, q, :], z[:, q, :], thrd[:, q:q + 1], sp[:, q, :],
                        OP.is_gt, OP.mult, accum_out=nslice)

                if CFG["th_pergroup"]:
                    thpl_g = spool.tile([128, 4], F32, tag="thplg")
                    nc.scalar.activation(thpl_g[:], minz_g, AF.Exp, bias=bneg4[:],
                                         scale=-1.0)
                    nc.vector.tensor_scalar(thpl_g[:], thpl_g[:], 1.0, None, OP.add)
                    nc.scalar.activation(thpl_g[:], thpl_g[:], AF.Ln)
                    nc.vector.tensor_tensor(out=thpl_g[:], in0=thpl_g[:],
                                            in1=hasm[:], op=OP.mult)
                    cmax_g = spool.tile([128, 4], F32, tag="cmaxg")
                    nc.vector.tensor_scalar(cmax_g[:], tcnt_g[:], 1.0, None, OP.max)
                    rec_g = spool.tile([128, 4], F32, tag="recg")
                    nc.vector.reciprocal(rec_g[:], cmax_g[:])
                    cm_g = spool.tile([128, 4], F32, tag="cmg")
                    nc.vector.tensor_scalar(cm_g[:], tcnt_g[:], 0.5, None, OP.is_gt)
                    nc.vector.tensor_tensor(out=thn_g[:], in0=thn_g[:], in1=rec_g[:],
                                            op=OP.mult)
                    nc.vector.tensor_tensor(out=thn_g[:], in0=thn_g[:], in1=cm_g[:],
                                            op=OP.mult)
                    nc.vector.tensor_tensor(out=thpl_g[:], in0=thpl_g[:],
                                            in1=thn_g[:], op=OP.add)
                    thsum_g = spool.tile([128, 1], F32, tag="thsumg")
                    nc.vector.tensor_reduce(thsum_g[:], thpl_g[:], axis=AX.X,
                                            op=OP.add)
                    if g == 0:
                        nc.vector.tensor_copy(thacc[:], thsum_g[:])
                    else:
                        nc.vector.tensor_tensor(out=thacc[:], in0=thacc[:],
                                                in1=thsum_g[:], op=OP.add)

            if not CFG["h_early"]:
                h_ps = h_path()

            # ---------------- th tail ----------------
            if CFG["th_pergroup"]:
                th_in = thacc
            else:
                thpl = smp.tile([128, KC], F32, tag="thpl")
                nc.scalar.activation(thpl[:], minz_all[:], AF.Exp, bias=bneg4[:],
                                     scale=-1.0)
                nc.vector.tensor_scalar(thpl[:], thpl[:], 1.0, None, OP.add)
                nc.scalar.activation(thpl[:], thpl[:], AF.Ln)
                hmall = smp.tile([128, KC], F32, tag="hmall")
                nc.vector.tensor_scalar(hmall[:], minz_all[:], -2.0, None, OP.is_lt)
                nc.vector.tensor_tensor(out=thpl[:], in0=thpl[:], in1=hmall[:],
                                        op=OP.mult)
                cmax2 = smp.tile([128, KC], F32, tag="cmax2")
                nc.vector.tensor_scalar(cmax2[:], tcnt_all[:], 1.0, None, OP.max)
                rec2 = smp.tile([128, KC], F32, tag="rec2")
                nc.vector.reciprocal(rec2[:], cmax2[:])
                cm2 = smp.tile([128, KC], F32, tag="cm2")
                nc.vector.tensor_scalar(cm2[:], tcnt_all[:], 0.5, None, OP.is_gt)
                nc.vector.tensor_tensor(out=thn_all[:], in0=thn_all[:], in1=rec2[:],
                                        op=OP.mult)
                nc.vector.tensor_tensor(out=thn_all[:], in0=thn_all[:], in1=cm2[:],
                                        op=OP.mult)
                nc.vector.tensor_tensor(out=thpl[:], in0=thpl[:], in1=thn_all[:],
                                        op=OP.add)
                th_in = smp.tile([128, 1], F32, tag="thsumc")
                nc.vector.tensor_reduce(th_in[:], thpl[:], axis=AX.X, op=OP.add)
            th_ps = tpp.tile([1, 1], F32, tag="tp", name="th_ps")
            nc.tensor.matmul(th_ps[:], th_in[:], onesf[:], start=True, stop=True)

            # ---------------- pack partials [1, 1024] ----------------
            # [0:256] M | [256:512] S | [512:768] TG | [768] th | [769] h
            pack = smp.tile([1, 1024], F32, tag="pack")
            nc.vector.memset(pack[:], 0.0)
            for m in range(2):
                mloc = smp.tile([128, 1], F32, tag=f"mloc{m}", name=f"mloc{m}")
                nc.vector.tensor_reduce(mloc[:], rowmax[m][:], axis=AX.X, op=OP.max)
                dd = smp.tile([128, NG], F32, tag=f"dd{m}", name=f"dd{m}")
                nc.vector.tensor_scalar(dd[:], rowmax[m][:], mloc[:], None,
                                        OP.subtract)
                nc.scalar.activation(dd[:], dd[:], AF.Exp)
                nc.vector.tensor_tensor(out=dd[:], in0=dd[:], in1=sexp[m][:],
                                        op=OP.mult)
                sloc = smp.tile([128, 1], F32, tag=f"sloc{m}", name=f"sloc{m}")
                nc.vector.tensor_reduce(sloc[:], dd[:], axis=AX.X, op=OP.add)
                tgl = smp.tile([128, 1], F32, tag=f"tgl{m}", name=f"tgl{m}")
                nc.vector.tensor_reduce(tgl[:], tgat[m][:], axis=AX.X, op=OP.add)
                for seg, srcv in ((0, mloc), (1, sloc), (2, tgl)):
                    pr_ps = tpp.tile([128, B], F32, tag="tp", name="pr_ps")
                    nc.tensor.transpose(pr_ps[0:1, 0:128], srcv[:], ident[:])
                    nc.scalar.copy(pack[0:1, seg * 256 + m * 128:
                                        seg * 256 + (m + 1) * 128],
                                   pr_ps[0:1, 0:128])
            nc.scalar.copy(pack[0:1, 768:769], th_ps[:])
            nc.scalar.copy(pack[0:1, 769:770], h_ps[:])

            # ---------------- AllGather + global merge ----------------
            ag_in = dram.tile([1, 1024], F32)
            ag_out = dram.tile([NCORES, 1024], F32)
            nc.sync.dma_start(out=ag_in[:], in_=pack[:])
            nc.gpsimd.collective_compute(
                "AllGather", OP.bypass, replica_groups=[list(range(NCORES))],
                ins=[ag_in[:].opt()], outs=[ag_out[:].opt()])
            ag = smp.tile([NCORES, 1024], F32, tag="ag")
            nc.sync.dma_start(out=ag[:], in_=ag_out[:])

            mg = smp.tile([NCORES, B], F32, tag="mg")
            nc.gpsimd.partition_all_reduce(mg[:], ag[:, 0:256], channels=NCORES,
                                           reduce_op=bass_isa.ReduceOp.max)
            dd2 = smp.tile([NCORES, B], F32, tag="dd2")
            nc.vector.tensor_tensor(out=dd2[:], in0=ag[:, 0:256], in1=mg[:],
                                    op=OP.subtract)
            nc.scalar.activation(dd2[:], dd2[:], AF.Exp)
            nc.vector.tensor_tensor(out=dd2[:], in0=dd2[:], in1=ag[:, 256:512],
                                    op=OP.mult)
            nc.vector.tensor_copy(ag[:, 256:512], dd2[:])
            addred = smp.tile([NCORES, 514], F32, tag="addred")
            nc.gpsimd.partition_all_reduce(addred[:], ag[:, 256:770], channels=NCORES,
                                           reduce_op=bass_isa.ReduceOp.add)

            lnS = smp.tile([1, B], F32, tag="lnS")
            nc.scalar.activation(lnS[:], addred[0:1, 0:256], AF.Ln)
            lp = smp.tile([1, B], F32, tag="lp")
            nc.vector.tensor_tensor(out=lp[:], in0=addred[0:1, 256:512],
                                    in1=mg[0:1, :], op=OP.subtract)
            nc.vector.tensor_tensor(out=lp[:], in0=lp[:], in1=lnS[:], op=OP.subtract)
            busum = smp.tile([1, 1], F32, tag="busum")
            nc.vector.tensor_reduce(busum[:], lp[:], axis=AX.X, op=OP.add)

            lossv = smp.tile([1, 1], F32, tag="lossv")
            nc.vector.tensor_scalar(lossv[:], busum[:], -1.0 / B, None, OP.mult)
            tmp1 = smp.tile([1, 1], F32, tag="tmp1")
            nc.vector.tensor_scalar(tmp1[:], addred[0:1, 512:513], 1.0 / C, None,
                                    OP.mult)
            nc.vector.tensor_tensor(out=lossv[:], in0=lossv[:], in1=tmp1[:], op=OP.add)
            nc.vector.tensor_scalar(tmp1[:], addred[0:1, 513:514],
                                    1.0 / (NCORES * B), None, OP.mult)
            nc.vector.tensor_tensor(out=lossv[:], in0=lossv[:], in1=tmp1[:], op=OP.add)
            nc.sync.dma_start(out=loss_d[:, :], in_=lossv[:])

    nc.compile()
    return nc


_NC = None


def _get_nc():
    global _NC
    if _NC is None:
        _NC = build()
    return _NC


def prepare_in_maps(inputs, targets, pos_mask, neg_mask, V):
    xT = np.ascontiguousarray(np.asarray(inputs, dtype=np.float32).T)
    tf = np.asarray(targets).astype(np.float32)
    posm = np.asarray(pos_mask).astype(np.uint8)
    negm = np.asarray(neg_mask).astype(np.uint8)
    V = np.asarray(V, dtype=np.float32)
    in_maps = []
    for i in range(NCORES):
        vT = np.ascontiguousarray(V[i * CL:(i + 1) * CL, :].T)
        ts = tf - i * CL
        in_maps.append({
            "xT": xT, "vT": vT,
            "trow": ts[None, :].copy(),
            "tcol": ts[:, None].copy(),
            "posm": posm, "negm": negm,
        })
    return in_maps


def run(in_maps, **kwargs):
    nc = _get_nc()
    return bass_utils.run_bass_kernel_spmd(nc, in_maps, core_ids=list(range(NCORES)),
                                           **kwargs)


def kernel(inputs, targets, pos_mask, neg_mask, V):
    in_maps = prepare_in_maps(inputs, targets, pos_mask, neg_mask, V)
    res = run(in_maps)
    outputs = np.concatenate([r["out"] for r in res.results], axis=1)
    loss = np.float32(res.results[0]["loss"][0, 0])
    return loss, outputs
